# revision 1
# baseline (speedup 1.0000x reference)
"""CKY kernel v5 (PE-shift DP). for 8x Trainium2 NeuronCores.

emissions [32,128,128,128] f32 -> logZ [32] f32 (CKY inside log-partition of
data = logsumexp(emissions, -1)).

Design:
 - Data parallel: 4 sentences/core on 8 cores.
 - DP in scaled probability space (chart stores exp(t - ALPHA*p - DELTA)):
   the per-width logsumexp recurrence becomes pure multiply+add in fp32.
 - Phase 1 loads only the upper triangle of data, 8 diagonals per DMA
   (2KB/partition lines), exp on ACT, per-diagonal label-sums on Pool
   (tensor_scalar accum) or DVE (segmented reduce).
 - Dual chart F2 lives in PSUM: PSUM access patterns are exempt from the
   SBUF partition-start alignment rule, so the DP can address partition
   offset w directly. Diag tables are per-group SBUF tiles so the DP
   overlaps with phase 1.
"""
import os
import sys

sys.path.insert(0, "/opt/trn_rl_repo")

import numpy as np

import concourse.bacc as bacc
import concourse.mybir as mybir
import concourse.tile as tile
from concourse import bass_utils
from concourse.ap import AP

ALPHA = 12.05
DELTA = -10.9
N = 128
M = 128
BC = 4
NCORES = 8
G = 8
NG = N // G
_f32 = mybir.dt.float32


def _build_cky(tc, out_dram, em_dram, red_mode="dve"):
    nc = tc.nc
    K1 = float(np.exp(-ALPHA - 2 * DELTA))

    with tc.tile_pool(name="pers", bufs=1) as pers, \
         tc.tile_pool(name="st", bufs=6) as stp, \
         tc.tile_pool(name="psum", bufs=2, space="PSUM") as psp:

        F1 = pers.tile([N, BC * N], _f32)
        F2 = pers.tile([N, BC * N], _f32)
        nc.gpsimd.memset(F1[:], 0.0)
        nc.gpsimd.memset(F2[:], 0.0)
        # per-group diag tables: Dg[g][s, b*G + c] = sum_m exp(em[b,s,s+8g+c,m]+DELTA)
        Dg = [pers.tile([N, BC * G], _f32, tag=f"dred{g}", name=f"dred{g}") for g in range(NG)]
        biasap = pers.tile([N, 1], _f32)
        nc.vector.memset(biasap[:], DELTA)

        # ---- phase 1 ----
        for g in range(NG):
            for b in range(BC):
                base = b * N * N * M
                st = stp.tile([N, G * M], _f32, tag="st")
                L = N - G * g
                ragged = (g == 0 and b == BC - 1)
                if ragged:
                    L = N - (G - 1)  # avoid reading past end of tensor
                    nc.gpsimd.memset(st[:], 0.0)
                src = AP(em_dram.tensor, base + G * g * M,
                         [[(N + 1) * M, L], [1, G * M]])
                nc.sync.dma_start(st[0:L, :], src)
                if ragged:
                    # fixup rows s = N-G+1 .. N-1: load (s, j=s..N-1, :) contiguous
                    for s in range(N - G + 1, N):
                        cnt = (N - s) * M
                        fsrc = AP(em_dram.tensor, base + (s * N + s) * M,
                                  [[cnt, 1], [1, cnt]])
                        nc.sync.dma_start(st[s:s + 1, 0:cnt], fsrc)
                LE = N if ragged else L  # exp/reduce coverage
                nc.scalar.activation(st[0:LE, :], st[0:LE, :],
                                     mybir.ActivationFunctionType.Exp,
                                     bias=biasap[0:LE, :], scale=1.0)
                if red_mode == "dve":
                    st3 = st.rearrange("s (c m) -> s c m", c=G)
                    dg3 = Dg[g].rearrange("s (c b) -> s c b", c=G)
                    nc.vector.reduce_sum(dg3[0:LE, :, b], st3[0:LE],
                                         axis=mybir.AxisListType.X)
                else:
                    for c in range(G):
                        col = c * BC + b
                        nc.gpsimd.tensor_scalar(
                            st[0:LE, c * M:(c + 1) * M],
                            st[0:LE, c * M:(c + 1) * M],
                            1.0, 0.0, mybir.AluOpType.mult, mybir.AluOpType.add,
                            accum_out=Dg[g][0:LE, col:col + 1])

        # ---- width-0 init (v-major layout: col v*BC+b) ----
        nc.vector.tensor_scalar_mul(F1[:, 0:BC], Dg[0][:, 0:BC], K1)
        nc.vector.tensor_scalar_mul(F2[:, (N - 1) * BC:N * BC],
                                    Dg[0][:, 0:BC], K1)

        # ---- DP ----
        # W[k, c] = 1 iff c == k + 128: sliced as matmul lhsT it shifts
        # partitions: (W[:, 128+d : 256+d].T @ X)[m, :] = X[m + d, :].
        Wm = pers.tile([N, 3 * N], _f32)
        nc.gpsimd.memset(Wm[:], 0.0)
        nc.gpsimd.affine_select(out=Wm[:], in_=Wm[:],
                                compare_op=mybir.AluOpType.not_equal,
                                fill=1.0, base=N,
                                pattern=[[-1, 3 * N]], channel_multiplier=1)

        prod = pers.tile([N, BC * N], _f32)
        red = pers.tile([N, BC], _f32)
        red2 = pers.tile([N, BC], _f32)
        t0 = pers.tile([N, BC], _f32)
        Dgv = [d.rearrange("s (c b) -> s c b", c=G) for d in Dg]

        for w in range(1, N):
            L = N - w
            dcol = Dgv[w // G][0:L, (w % G), :]
            # v=0 term: F2[s+w, col n-w] = F1[s+1, col w-1] -> PE shift by 1
            ps1 = psp.tile([N, BC], _f32, tag="ps1", name=f"ps1_{w}")
            nc.tensor.matmul(ps1[:], Wm[:, N + 1:2 * N + 1],
                             F1[:, (w - 1) * BC:w * BC])
            if w >= 2:
                # v in [1, w): psS[m, (v,b)] = F2[m + w, ((n-w+v), b)]
                psS = psp.tile([N, BC * N], _f32, tag="psS", name=f"psS_{w}")
                nc.tensor.matmul(psS[:, BC:w * BC], Wm[:, N + w:2 * N + w],
                                 F2[:, (N - w + 1) * BC:N * BC])
                nc.vector.tensor_mul(prod[0:L, BC:w * BC],
                                     F1[0:L, BC:w * BC],
                                     psS[0:L, BC:w * BC])
                pr3 = prod.rearrange("s (v b) -> s b v", b=BC)
                nc.vector.reduce_sum(red[0:L, :], pr3[0:L, :, 1:w],
                                     axis=mybir.AxisListType.X)
            nc.vector.tensor_mul(t0[0:L, :], F1[0:L, 0:BC], ps1[0:L, :])
            if w >= 2:
                nc.vector.tensor_add(red2[0:L, :], red[0:L, :], t0[0:L, :])
            else:
                nc.vector.tensor_copy(red2[0:L, :], t0[0:L, :])
            nc.vector.tensor_mul(F1[0:L, w * BC:(w + 1) * BC],
                                 red2[0:L, :], dcol)
            if w < N - 1:
                # dual-chart column write at partition offset w, via DMA
                nc.sync.dma_start(F2[w:N, (N - 1 - w) * BC:(N - w) * BC],
                                  F1[0:L, w * BC:(w + 1) * BC])

        nc.sync.dma_start(out_dram[:], F1[0:1, (N - 1) * BC:N * BC])


_CACHE: dict = {}


def _get_nc():
    if "nc" not in _CACHE:
        red_mode = os.environ.get("CKY_RED_MODE", "dve")
        nc = bacc.Bacc("TRN2", target_bir_lowering=False, debug=False,
                       enable_asserts=False, num_devices=NCORES)
        em = nc.dram_tensor("emissions", [BC, N, N, M], _f32,
                            kind="ExternalInput")
        out = nc.dram_tensor("out", [BC], _f32, kind="ExternalOutput")
        with tile.TileContext(nc) as tc:
            _build_cky(tc, out.ap(), em.ap(), red_mode=red_mode)
        nc.compile()
        _CACHE["nc"] = nc
    return _CACHE["nc"]


def _run(emissions, **spmd_kwargs):
    emissions = np.ascontiguousarray(emissions, dtype=np.float32)
    assert emissions.shape == (BC * NCORES, N, N, M)
    nc = _get_nc()
    in_maps = [{"emissions": emissions[c * BC:(c + 1) * BC]}
               for c in range(NCORES)]
    res = bass_utils.run_bass_kernel_spmd(nc, in_maps,
                                          core_ids=list(range(NCORES)),
                                          **spmd_kwargs)
    outs = np.concatenate([res.results[c]["out"] for c in range(NCORES)])
    logz = np.log(outs.astype(np.float64)) + (ALPHA * N + DELTA)
    return logz.astype(np.float32), res


def kernel(emissions):
    logz, _ = _run(emissions)
    return logz


def kernel_traced(emissions):
    """Like kernel() but with NTFF tracing; returns (logZ, BassKernelResults)."""
    return _run(emissions, trace=True)


def kernel_bench(emissions, iters=10):
    """Time the on-device execution: jit built once, inputs device-resident.

    Returns (logZ, best_seconds_per_call).
    """
    import time
    import jax
    from jax.sharding import Mesh, PartitionSpec
    from jax.experimental.shard_map import shard_map
    import concourse.mybir as mybir
    from concourse import bass2jax

    emissions = np.ascontiguousarray(emissions, dtype=np.float32)
    nc = _get_nc()
    bass2jax.install_neuronx_cc_hook()

    in_names, out_names, out_avals, zero_outs = [], [], [], []
    for alloc in nc.m.functions[0].allocations:
        if not isinstance(alloc, mybir.MemoryLocationSet):
            continue
        name = alloc.memorylocations[0].name
        if alloc.kind == "ExternalInput":
            if nc.partition_id_tensor is None or \
                    name != nc.partition_id_tensor.name:
                in_names.append(name)
        elif alloc.kind == "ExternalOutput":
            out_names.append(name)
            shape = tuple(alloc.tensor_shape)
            dtype = mybir.dt.np(alloc.dtype)
            out_avals.append(jax.core.ShapedArray(shape, dtype))
            zero_outs.append(np.zeros(shape, dtype))
    n_params = len(in_names)
    all_in_names = in_names + out_names
    if nc.partition_id_tensor is not None:
        all_in_names = all_in_names + [nc.partition_id_tensor.name]

    def _body(*args):
        operands = list(args)
        if nc.partition_id_tensor is not None:
            operands.append(bass2jax.partition_id_tensor())
        outs = bass2jax._bass_exec_p.bind(
            *operands, out_avals=tuple(out_avals), in_names=tuple(all_in_names),
            out_names=tuple(out_names), lowering_input_output_aliases=(),
            sim_require_finite=True, sim_require_nnan=True, nc=nc)
        return tuple(outs)

    devices = jax.devices()[:NCORES]
    mesh = Mesh(np.asarray(devices), ("core",))
    fn = jax.jit(shard_map(_body, mesh=mesh,
                           in_specs=(PartitionSpec("core"),) * (n_params + 1),
                           out_specs=(PartitionSpec("core"),),
                           check_rep=False),
                 keep_unused=True)
    x = jax.device_put(emissions)  # [32, N, N, M] sharded? put replicated-ish
    sharding = jax.sharding.NamedSharding(mesh, PartitionSpec("core"))
    x = jax.device_put(emissions, sharding)
    z = jax.device_put(np.zeros((NCORES * BC,), np.float32), sharding)
    out = fn(x, z)[0]
    out.block_until_ready()  # warm
    best = float("inf")
    for _ in range(iters):
        t0 = time.perf_counter()
        out = fn(x, z)[0]
        out.block_until_ready()
        best = min(best, time.perf_counter() - t0)
    outs = np.asarray(out)
    logz = np.log(outs.astype(np.float64)) + (ALPHA * N + DELTA)
    return logz.astype(np.float32), best

